# revision 21
# baseline (speedup 1.0000x reference)
"""Distributed Trainium2 kernel for nn_Attention_14697378086932.

Head-sharded (tensor-parallel) multi-head attention over 8 NeuronCores:
each core computes 2 of the 16 heads end-to-end.

Per core c:
  - QKV projections:  Q^T = Wq_c^T @ X^T  (f32r matmuls, contraction on
    hidden dim), giving Q^T/K^T/V^T in [128 local channels, 4096 tokens]
    layout (d-on-partitions), which is exactly the layout the scores
    matmul wants.
  - rotary: only global channels 0..63 are rotated (reference quirk), i.e.
    local channels 0..63 of core 0.  All cores run the same graph; cores
    1..7 receive cos=1/sin=0 so their "rotation" is the identity.
    rotate_half is a fixed 64x64 permutation matrix applied on the PE.
  - attention (per batch, per local head, flash-style over 128-token key
    chunks): S^T = K Q^T (f32r), P^T = exp(S^T) on the scalar engine
    (no max subtraction -- logits are bounded, f32/bf16 exp is safe),
    O^T = [V | 1]^T @ P^T (bf16) which yields the softmax denominator as
    a free 65th row.  Normalize with a reciprocal multiply.
  - output projection: partial = O_loc @ Wo_c (bf16), DMA'd out per
    128-token chunk.
Host sums the 8 partial outputs and adds bo.
"""
import sys
import types

sys.path.insert(0, "/opt/trn_rl_repo")

import numpy as np
import ml_dtypes

import concourse.bass as bass
import concourse.mybir as mybir
from concourse import bacc
from concourse.bass import ts, ds
from concourse.tile import TileContext
from concourse.masks import make_identity
from concourse.bass_utils import run_bass_kernel_spmd

F32 = mybir.dt.float32
F32R = mybir.dt.float32r
BF16 = mybir.dt.bfloat16

P = 128          # partitions / local channels per core
HID = 1024       # hidden
NT = 4096        # total tokens (batch 2 x 2048)
NB = 2048        # tokens per batch
HD = 64          # head dim
N_CORES = 8

_NC_CACHE = None


def build_nc():
    nc = bacc.Bacc("TRN2")

    xt = nc.declare_dram_parameter("xt", [HID, NT], F32R, isOutput=False)
    wq = nc.declare_dram_parameter("wq", [HID, P], F32R, isOutput=False)
    wk = nc.declare_dram_parameter("wk", [HID, P], F32R, isOutput=False)
    wv = nc.declare_dram_parameter("wv", [HID, P], F32R, isOutput=False)
    wo = nc.declare_dram_parameter("wo", [P, HID], BF16, isOutput=False)
    bia = nc.declare_dram_parameter("bias", [P, 3], F32, isOutput=False)
    cos = nc.declare_dram_parameter("cos", [HD, NT], F32, isOutput=False)
    sin = nc.declare_dram_parameter("sin", [HD, NT], F32, isOutput=False)
    rmat = nc.declare_dram_parameter("rmat", [HD, HD], F32R, isOutput=False)
    out = nc.declare_dram_parameter("out", [NT, HID], F32, isOutput=True)

    xt_r = xt[:].rearrange("(o p) n -> p o n", p=P)      # [128, 8, 4096]
    wq_r = wq[:].rearrange("(o p) m -> p o m", p=P)      # [128, 8, 128]
    wk_r = wk[:].rearrange("(o p) m -> p o m", p=P)
    wv_r = wv[:].rearrange("(o p) m -> p o m", p=P)

    with TileContext(nc) as tc:
        with tc.tile_pool(name="consts", bufs=1) as consts, \
             tc.tile_pool(name="big", bufs=1) as big:
            wqs = consts.tile([P, 8, P], F32R)
            wks = consts.tile([P, 8, P], F32R)
            wvs = consts.tile([P, 8, P], F32R)
            nc.sync.dma_start(wqs, wq_r)
            nc.sync.dma_start(wks, wk_r)
            nc.sync.dma_start(wvs, wv_r)
            wos = consts.tile([P, HID], BF16)
            nc.sync.dma_start(wos, wo[:])
            bias_t = consts.tile([P, 3], F32)
            nc.sync.dma_start(bias_t, bia[:])
            rmat_t = consts.tile([HD, HD], F32R)
            nc.sync.dma_start(rmat_t, rmat[:])
            ident = consts.tile([P, P], F32)
            make_identity(nc, ident)

            Qt = big.tile([P, NT], F32R)     # Q^T (local channels x tokens)
            Kt = big.tile([P, NT], F32R)
            Vt = big.tile([P, NT], F32)      # V^T, pre-transpose
            Ot = big.tile([P, NT], BF16)     # normalized attention out^T
            # V in natural [token, channel] layout + ones column, per head,
            # per 128-token key chunk: [128 tok, 32 chunks, 64 V | 1 | pad]
            VaugA = big.tile([P, 32, 66], BF16)
            VaugB = big.tile([P, 32, 66], BF16)
            nc.vector.memset(VaugA[:, :, 64:66], 1.0)
            nc.vector.memset(VaugB[:, :, 64:66], 1.0)

            # ---------------- Phase A: QKV projections + rope + V transpose
            with tc.tile_pool(name="xtp", bufs=3) as xtp, \
                 tc.tile_pool(name="ropet", bufs=2) as ropet, \
                 tc.tile_pool(name="trig", bufs=1) as trig, \
                 tc.tile_pool(name="psA", bufs=4, space="PSUM") as psA:
                cos_t = trig.tile([HD, NT], F32)
                sin_t = trig.tile([HD, NT], F32)
                nc.sync.dma_start(cos_t, cos[:])
                nc.sync.dma_start(sin_t, sin[:])
                for cc in range(8):   # 512-token chunks
                    sl = ts(cc, 512)
                    xtt = xtp.tile([P, 8, 512], F32R, tag="xt")
                    nc.sync.dma_start(xtt, xt_r[:, :, sl])
                    for wt, bidx, dst in ((wqs, 0, Qt), (wks, 1, Kt), (wvs, 2, Vt)):
                        ps = psA.tile([P, 512], F32, tag="ps")
                        for o in range(8):
                            nc.tensor.matmul(ps, wt[:, o], xtt[:, o],
                                             start=(o == 0), stop=(o == 7))
                        nc.scalar.activation(
                            dst[:, sl], ps,
                            mybir.ActivationFunctionType.Identity,
                            bias=bias_t[:, bidx:bidx + 1])
                    # rope on first 64 local channels of Q and K
                    for t in (Qt, Kt):
                        psr = psA.tile([P, 512], F32, tag="ps")
                        nc.tensor.matmul(psr[0:HD], rmat_t,
                                         t[0:HD, sl],
                                         start=True, stop=True)
                        tmp = ropet.tile([HD, 512], F32, tag="tmp")
                        nc.vector.tensor_tensor(tmp, psr[0:HD], sin_t[:, sl],
                                                mybir.AluOpType.mult)
                        nc.vector.tensor_tensor(t[0:HD, sl], t[0:HD, sl],
                                                cos_t[:, sl], mybir.AluOpType.mult)
                        nc.vector.tensor_tensor(t[0:HD, sl], t[0:HD, sl], tmp,
                                                mybir.AluOpType.add)
                    # V transpose into per-head natural layout (+ones col)
                    for s in range(4):
                        kc = cc * 4 + s
                        pst = psA.tile([P, 512], F32, tag="ps")
                        nc.tensor.transpose(pst[:, 0:P], Vt[:, ts(kc, P)], ident)
                        nc.vector.tensor_copy(VaugA[:, kc, 0:HD], pst[:, 0:HD])
                        nc.vector.tensor_copy(VaugB[:, kc, 0:HD], pst[:, HD:P])

            # ---------------- Phase B: attention + output projection
            # Both local heads are processed together per (batch, nq-block):
            # their S^T matmuls contract only 64 partitions each, so head A
            # (rows 0..63) and head B (rows 64..127) run CONCURRENTLY in
            # different PE row-groups (tile_position auto-derived from the
            # APs' base partitions).  The exp stream on ScalarE is the
            # bottleneck; PE has enough slack that the output projection of
            # the previous block can borrow the freed O-psum slots.
            with tc.tile_pool(name="ptp", bufs=10) as ptp, \
                 tc.tile_pool(name="osb", bufs=3) as osb, \
                 tc.tile_pool(name="nrm", bufs=2) as nrm, \
                 tc.tile_pool(name="spSA", bufs=1, space="PSUM") as spSA, \
                 tc.tile_pool(name="spSB", bufs=1, space="PSUM") as spSB, \
                 tc.tile_pool(name="spOA", bufs=1, space="PSUM") as spOA, \
                 tc.tile_pool(name="spOB", bufs=1, space="PSUM") as spOB:

                def oproj(q0):
                    # output projection for 1024 tokens (both heads), psum
                    # borrowed from the O slots the normalize just released
                    for tch in range(8):
                        t0 = q0 + tch * P
                        pool = spOA if tch % 2 == 0 else spOB
                        Pps = pool.tile([P, 1024], F32, tag="O")
                        for hf in range(2):
                            nc.tensor.matmul(
                                Pps[:, ts(hf, 512)],
                                Ot[:, t0:t0 + P],
                                wos[:, ts(hf, 512)],
                                start=True, stop=True)
                        ost = osb.tile([P, HID], F32, tag="ost")
                        nc.any.tensor_copy(ost, Pps)
                        nc.sync.dma_start(out[t0:t0 + P, :], ost)

                prev_q0 = None
                for b in range(2):
                    for nqb in range(2):
                        q0 = b * NB + nqb * 1024

                        def s_exp(i, hlo, spool, ptag):
                            k0 = b * NB + i * P
                            Sps = spool.tile([P, 1024], F32, tag="S")
                            for hf in range(2):
                                nc.tensor.matmul(
                                    Sps[:, ts(hf, 512)],
                                    Kt[hlo:hlo + HD, k0:k0 + P],
                                    Qt[hlo:hlo + HD, ds(q0 + hf * 512, 512)],
                                    start=True, stop=True)
                            Pt = ptp.tile([P, 1024], BF16, tag=ptag)
                            nc.scalar.activation(
                                Pt, Sps, mybir.ActivationFunctionType.Exp)
                            return Pt

                        def pv(i, Pt, Vaug, Ops):
                            kc = b * 16 + i
                            for hf in range(2):
                                nc.tensor.matmul(
                                    Ops[:, ts(hf, 512)],
                                    Vaug[:, kc, 0:HD + 1],
                                    Pt[:, ts(hf, 512)],
                                    start=(i == 0), stop=(i == 15),
                                    skip_group_check=True)

                        # software pipeline: S^T/exp run DEPTH chunks ahead
                        # of PV, so the PE never queues behind the exp
                        # stream and the previous block's output projection
                        # (which must wait for that block's normalize) gets
                        # enough runway to never stall the PE FIFO.
                        DEPTH = 8
                        pend = []
                        for i in range(DEPTH):
                            pend.append((s_exp(i, 0, spSA, "PA"),
                                         s_exp(i, HD, spSB, "PB")))
                        if prev_q0 is not None:
                            oproj(prev_q0)
                        # allocate AFTER oproj(prev) so the O-slot reuse
                        # chain is Ops(j) -> oproj(j) -> Ops(j+1)
                        OpsA = spOA.tile([HD + 1, 1024], F32, tag="O")
                        OpsB = spOB.tile([HD + 1, 1024], F32, tag="O")
                        for i in range(DEPTH, 16):
                            na = s_exp(i, 0, spSA, "PA")
                            nb = s_exp(i, HD, spSB, "PB")
                            pa, pb = pend.pop(0)
                            pv(i - DEPTH, pa, VaugA, OpsA)
                            pv(i - DEPTH, pb, VaugB, OpsB)
                            pend.append((na, nb))
                        for j, (pa, pb) in enumerate(pend):
                            pv(16 - DEPTH + j, pa, VaugA, OpsA)
                            pv(16 - DEPTH + j, pb, VaugB, OpsB)

                        # normalize: copy out of PSUM right away so the O
                        # banks free up, then rows 0..63 / row 64 from SBUF
                        # (bf16 is plenty: the numerator is already bf16)
                        for hlo, Ops in ((0, OpsA), (HD, OpsB)):
                            osum = nrm.tile([HD + 1, 1024], BF16, tag="osum")
                            rc = nrm.tile([1, 1024], BF16, tag="rc")
                            rcb = nrm.tile([HD, 1024], BF16, tag="rcb")
                            with nc.allow_low_precision(
                                    reason="numerator is bf16 anyway"):
                                nc.vector.tensor_copy(osum, Ops)
                                nc.vector.reciprocal(rc, osum[HD:HD + 1, :])
                            nc.gpsimd.partition_broadcast(rcb, rc)
                            nc.vector.tensor_tensor(
                                Ot[hlo:hlo + HD, ds(q0, 1024)],
                                osum[0:HD, :],
                                rcb,
                                mybir.AluOpType.mult)
                        prev_q0 = q0
                oproj(prev_q0)

    nc.compile()
    return nc


def _get_nc():
    global _NC_CACHE
    if _NC_CACHE is None:
        _NC_CACHE = build_nc()
    return _NC_CACHE


def shard_inputs(x, rope_cos, rope_sin, Wq, bq, Wk, bk, Wv, bv, Wo, bo):
    """Build per-core input maps."""
    xt = np.ascontiguousarray(x.reshape(NT, HID).T).astype(np.float32)
    cosT = np.ascontiguousarray(rope_cos.reshape(NT, HD).T).astype(np.float32)
    sinT = np.ascontiguousarray(rope_sin.reshape(NT, HD).T).astype(np.float32)
    cos_id = np.ones((HD, NT), np.float32)
    sin_id = np.zeros((HD, NT), np.float32)
    # rotate_half as matrix R: out = R @ t, R[2i,2i+1]=-1, R[2i+1,2i]=+1.
    # matmul computes lhsT.T @ rhs, so pass R.T.
    R = np.zeros((HD, HD), np.float32)
    idx = np.arange(0, HD, 2)
    R[idx, idx + 1] = -1.0
    R[idx + 1, idx] = 1.0
    rmat = np.ascontiguousarray(R.T)

    in_maps = []
    for c in range(N_CORES):
        lo, hi = c * P, (c + 1) * P
        in_maps.append({
            "xt": xt,
            "wq": np.ascontiguousarray(Wq[:, lo:hi]).astype(np.float32),
            "wk": np.ascontiguousarray(Wk[:, lo:hi]).astype(np.float32),
            "wv": np.ascontiguousarray(Wv[:, lo:hi]).astype(np.float32),
            "wo": np.ascontiguousarray(Wo[lo:hi, :]).astype(ml_dtypes.bfloat16),
            "bias": np.ascontiguousarray(
                np.stack([bq[lo:hi], bk[lo:hi], bv[lo:hi]], axis=1)
            ).astype(np.float32),
            "cos": cosT if c == 0 else cos_id,
            "sin": sinT if c == 0 else sin_id,
            "rmat": rmat,
        })
    return in_maps


def run_device(inputs, trace=False, **kw):
    nc = _get_nc()
    in_maps = shard_inputs(**inputs)
    res = run_bass_kernel_spmd(nc, in_maps, core_ids=list(range(N_CORES)),
                               trace=trace, **kw)
    return res


def gather(res, bo):
    acc = res.results[0]["out"].astype(np.float32).copy()
    for c in range(1, N_CORES):
        acc += res.results[c]["out"]
    acc += bo[None, :].astype(np.float32)
    return acc.reshape(2, NB, HID)


def kernel(**inputs):
    res = run_device(inputs, trace=False)
    return gather(res, np.asarray(inputs["bo"], np.float32))


# revision 22
# speedup vs baseline: 1.0972x; 1.0972x over previous
"""Distributed Trainium2 kernel for nn_Attention_14697378086932.

Head-sharded (tensor-parallel) multi-head attention over 8 NeuronCores:
each core computes 2 of the 16 heads end-to-end.

Per core c:
  - QKV projections:  Q^T = Wq_c^T @ X^T  (f32r matmuls, contraction on
    hidden dim), giving Q^T/K^T/V^T in [128 local channels, 4096 tokens]
    layout (d-on-partitions), which is exactly the layout the scores
    matmul wants.
  - rotary: only global channels 0..63 are rotated (reference quirk), i.e.
    local channels 0..63 of core 0.  All cores run the same graph; cores
    1..7 receive cos=1/sin=0 so their "rotation" is the identity.
    rotate_half is a fixed 64x64 permutation matrix applied on the PE.
  - attention (per batch, per local head, flash-style over 128-token key
    chunks): S^T = K Q^T (f32r), P^T = exp(S^T) on the scalar engine
    (no max subtraction -- logits are bounded, f32/bf16 exp is safe),
    O^T = [V | 1]^T @ P^T (bf16) which yields the softmax denominator as
    a free 65th row.  Normalize with a reciprocal multiply.
  - output projection: partial = O_loc @ Wo_c (bf16), DMA'd out per
    128-token chunk.
Host sums the 8 partial outputs and adds bo.
"""
import sys
import types

sys.path.insert(0, "/opt/trn_rl_repo")

import numpy as np
import ml_dtypes

import concourse.bass as bass
import concourse.mybir as mybir
from concourse import bacc
from concourse.bass import ts, ds
from concourse.tile import TileContext
from concourse.masks import make_identity
from concourse.bass_utils import run_bass_kernel_spmd

F32 = mybir.dt.float32
F32R = mybir.dt.float32r
BF16 = mybir.dt.bfloat16

P = 128          # partitions / local channels per core
HID = 1024       # hidden
NT = 4096        # total tokens (batch 2 x 2048)
NB = 2048        # tokens per batch
HD = 64          # head dim
N_CORES = 8

_NC_CACHE = None


def build_nc():
    nc = bacc.Bacc("TRN2")

    xt = nc.declare_dram_parameter("xt", [HID, NT], F32R, isOutput=False)
    wq = nc.declare_dram_parameter("wq", [HID, P], F32R, isOutput=False)
    wk = nc.declare_dram_parameter("wk", [HID, P], F32R, isOutput=False)
    wv = nc.declare_dram_parameter("wv", [HID, P], F32R, isOutput=False)
    wo = nc.declare_dram_parameter("wo", [P, HID], BF16, isOutput=False)
    bia = nc.declare_dram_parameter("bias", [P, 3], F32, isOutput=False)
    cos = nc.declare_dram_parameter("cos", [HD, NT], F32, isOutput=False)
    sin = nc.declare_dram_parameter("sin", [HD, NT], F32, isOutput=False)
    rmat = nc.declare_dram_parameter("rmat", [HD, HD], F32R, isOutput=False)
    out = nc.declare_dram_parameter("out", [NT, HID], F32, isOutput=True)

    xt_r = xt[:].rearrange("(o p) n -> p o n", p=P)      # [128, 8, 4096]
    wq_r = wq[:].rearrange("(o p) m -> p o m", p=P)      # [128, 8, 128]
    wk_r = wk[:].rearrange("(o p) m -> p o m", p=P)
    wv_r = wv[:].rearrange("(o p) m -> p o m", p=P)

    with TileContext(nc) as tc:
        with tc.tile_pool(name="consts", bufs=1) as consts, \
             tc.tile_pool(name="big", bufs=1) as big:
            wqs = consts.tile([P, 8, P], F32R)
            wks = consts.tile([P, 8, P], F32R)
            wvs = consts.tile([P, 8, P], F32R)
            nc.sync.dma_start(wqs, wq_r)
            nc.sync.dma_start(wks, wk_r)
            nc.sync.dma_start(wvs, wv_r)
            wos = consts.tile([P, HID], BF16)
            nc.sync.dma_start(wos, wo[:])
            bias_t = consts.tile([P, 3], F32)
            nc.sync.dma_start(bias_t, bia[:])
            rmat_t = consts.tile([HD, HD], F32R)
            nc.sync.dma_start(rmat_t, rmat[:])
            ident = consts.tile([P, P], F32)
            make_identity(nc, ident)

            Qt = big.tile([P, NT], F32R)     # Q^T (local channels x tokens)
            Kt = big.tile([P, NT], F32R)
            Vt = big.tile([P, NT], F32)      # V^T, pre-transpose
            Ot = big.tile([P, NT], BF16)     # normalized attention out^T
            # V in natural [token, channel] layout + ones column, per head,
            # per 128-token key chunk: [128 tok, 32 chunks, 64 V | 1 | pad]
            VaugA = big.tile([P, 32, 66], BF16)
            VaugB = big.tile([P, 32, 66], BF16)
            nc.vector.memset(VaugA[:, :, 64:66], 1.0)
            nc.vector.memset(VaugB[:, :, 64:66], 1.0)

            # ---------------- Phase A: QKV projections + rope + V transpose
            with tc.tile_pool(name="xtp", bufs=3) as xtp, \
                 tc.tile_pool(name="ropet", bufs=2) as ropet, \
                 tc.tile_pool(name="trig", bufs=1) as trig, \
                 tc.tile_pool(name="psA", bufs=4, space="PSUM") as psA:
                cos_t = trig.tile([HD, NT], F32)
                sin_t = trig.tile([HD, NT], F32)
                nc.sync.dma_start(cos_t, cos[:])
                nc.sync.dma_start(sin_t, sin[:])
                for cc in range(8):   # 512-token chunks
                    sl = ts(cc, 512)
                    xtt = xtp.tile([P, 8, 512], F32R, tag="xt")
                    nc.sync.dma_start(xtt, xt_r[:, :, sl])
                    for wt, bidx, dst in ((wqs, 0, Qt), (wks, 1, Kt), (wvs, 2, Vt)):
                        ps = psA.tile([P, 512], F32, tag="ps")
                        for o in range(8):
                            nc.tensor.matmul(ps, wt[:, o], xtt[:, o],
                                             start=(o == 0), stop=(o == 7))
                        nc.scalar.activation(
                            dst[:, sl], ps,
                            mybir.ActivationFunctionType.Identity,
                            bias=bias_t[:, bidx:bidx + 1])
                    # rope on first 64 local channels of Q and K
                    for t in (Qt, Kt):
                        psr = psA.tile([P, 512], F32, tag="ps")
                        nc.tensor.matmul(psr[0:HD], rmat_t,
                                         t[0:HD, sl],
                                         start=True, stop=True)
                        tmp = ropet.tile([HD, 512], F32, tag="tmp")
                        nc.vector.tensor_tensor(tmp, psr[0:HD], sin_t[:, sl],
                                                mybir.AluOpType.mult)
                        nc.vector.tensor_tensor(t[0:HD, sl], t[0:HD, sl],
                                                cos_t[:, sl], mybir.AluOpType.mult)
                        nc.vector.tensor_tensor(t[0:HD, sl], t[0:HD, sl], tmp,
                                                mybir.AluOpType.add)
                    # V transpose into per-head natural layout (+ones col)
                    for s in range(4):
                        kc = cc * 4 + s
                        pst = psA.tile([P, 512], F32, tag="ps")
                        nc.tensor.transpose(pst[:, 0:P], Vt[:, ts(kc, P)], ident)
                        nc.vector.tensor_copy(VaugA[:, kc, 0:HD], pst[:, 0:HD])
                        nc.vector.tensor_copy(VaugB[:, kc, 0:HD], pst[:, HD:P])

            # ---------------- Phase B: attention + output projection
            # One local head per block, S double-buffered so the exp stream
            # on ScalarE (the bottleneck) never waits on S^T latency.  The
            # output projection of a finished (b, nqb) token range runs in a
            # dedicated psum pool and is INJECTED into the middle of the
            # following blocks, well after its normalize has completed, so
            # it never stalls the PE FIFO.
            with tc.tile_pool(name="ptp", bufs=4) as ptp, \
                 tc.tile_pool(name="osb", bufs=3) as osb, \
                 tc.tile_pool(name="nrm", bufs=2) as nrm, \
                 tc.tile_pool(name="spS", bufs=2, space="PSUM") as spS, \
                 tc.tile_pool(name="spO", bufs=1, space="PSUM") as spO, \
                 tc.tile_pool(name="spP", bufs=1, space="PSUM") as spP:

                def oproj_tile(q0, tch):
                    # output projection of one 128-token chunk (both heads)
                    t0 = q0 + tch * P
                    Pps = spP.tile([P, 1024], F32, tag="oproj")
                    for hf in range(2):
                        nc.tensor.matmul(
                            Pps[:, ts(hf, 512)],
                            Ot[:, t0:t0 + P],
                            wos[:, ts(hf, 512)],
                            start=True, stop=True)
                    ost = osb.tile([P, HID], F32, tag="ost")
                    nc.any.tensor_copy(ost, Pps)
                    nc.sync.dma_start(out[t0:t0 + P, :], ost)

                # (q0, tch) work items for output projection, produced as
                # blocks complete, consumed at injection points
                oproj_queue = []
                blocks = [(b, nqb, h)
                          for b in range(2) for nqb in range(2)
                          for h in range(2)]
                for bi, (b, nqb, h) in enumerate(blocks):
                    q0 = b * NB + nqb * 1024
                    hlo = h * HD
                    Vaug = VaugA if h == 0 else VaugB
                    Ops = spO.tile([HD + 1, 1024], F32, tag="O")

                    def s_exp(i):
                        k0 = b * NB + i * P
                        Sps = spS.tile([P, 1024], F32, tag="S")
                        for hf in range(2):
                            nc.tensor.matmul(
                                Sps[:, ts(hf, 512)],
                                Kt[hlo:hlo + HD, k0:k0 + P],
                                Qt[hlo:hlo + HD, ds(q0 + hf * 512, 512)],
                                start=True, stop=True)
                        Pt = ptp.tile([P, 1024], BF16, tag="P")
                        nc.scalar.activation(
                            Pt, Sps, mybir.ActivationFunctionType.Exp)
                        return Pt

                    def pv(i, Pt):
                        kc = b * 16 + i
                        for hf in range(2):
                            nc.tensor.matmul(
                                Ops[:, ts(hf, 512)],
                                Vaug[:, kc, 0:HD + 1],
                                Pt[:, ts(hf, 512)],
                                start=(i == 0), stop=(i == 15),
                                skip_group_check=True)

                    # depth-2 software pipeline; inject one oproj chunk
                    # every other chunk once the source tokens' normalize
                    # is safely in the past (>= 2 blocks = 32 chunks ago)
                    pend = []
                    for i in range(16):
                        if i >= 2:
                            pv(i - 2, pend.pop(0))
                        pend.append(s_exp(i))
                        if i % 2 == 0 and oproj_queue and bi >= 2:
                            src = oproj_queue[0]
                            if src[2] <= bi - 2:
                                oproj_queue.pop(0)
                                oproj_tile(src[0], src[1])
                    pv(14, pend.pop(0))
                    pv(15, pend.pop(0))

                    # normalize: copy out of PSUM right away so the O bank
                    # frees up, then rows 0..63 / row 64 from SBUF
                    osum = nrm.tile([HD + 1, 1024], F32, tag="osum")
                    nc.vector.tensor_copy(osum, Ops)
                    rc = nrm.tile([1, 1024], F32, tag="rc")
                    nc.vector.reciprocal(rc, osum[HD:HD + 1, :])
                    rcb = nrm.tile([HD, 1024], F32, tag="rcb")
                    nc.gpsimd.partition_broadcast(rcb, rc)
                    nc.vector.tensor_tensor(
                        Ot[hlo:hlo + HD, ds(q0, 1024)],
                        osum[0:HD, :],
                        rcb,
                        mybir.AluOpType.mult)
                    if h == 1:
                        for tch in range(8):
                            oproj_queue.append((q0, tch, bi))
                # drain remaining output-projection work
                for q0_, tch_, _ in oproj_queue:
                    oproj_tile(q0_, tch_)

    nc.compile()
    return nc


def _get_nc():
    global _NC_CACHE
    if _NC_CACHE is None:
        _NC_CACHE = build_nc()
    return _NC_CACHE


def shard_inputs(x, rope_cos, rope_sin, Wq, bq, Wk, bk, Wv, bv, Wo, bo):
    """Build per-core input maps."""
    xt = np.ascontiguousarray(x.reshape(NT, HID).T).astype(np.float32)
    cosT = np.ascontiguousarray(rope_cos.reshape(NT, HD).T).astype(np.float32)
    sinT = np.ascontiguousarray(rope_sin.reshape(NT, HD).T).astype(np.float32)
    cos_id = np.ones((HD, NT), np.float32)
    sin_id = np.zeros((HD, NT), np.float32)
    # rotate_half as matrix R: out = R @ t, R[2i,2i+1]=-1, R[2i+1,2i]=+1.
    # matmul computes lhsT.T @ rhs, so pass R.T.
    R = np.zeros((HD, HD), np.float32)
    idx = np.arange(0, HD, 2)
    R[idx, idx + 1] = -1.0
    R[idx + 1, idx] = 1.0
    rmat = np.ascontiguousarray(R.T)

    in_maps = []
    for c in range(N_CORES):
        lo, hi = c * P, (c + 1) * P
        in_maps.append({
            "xt": xt,
            "wq": np.ascontiguousarray(Wq[:, lo:hi]).astype(np.float32),
            "wk": np.ascontiguousarray(Wk[:, lo:hi]).astype(np.float32),
            "wv": np.ascontiguousarray(Wv[:, lo:hi]).astype(np.float32),
            "wo": np.ascontiguousarray(Wo[lo:hi, :]).astype(ml_dtypes.bfloat16),
            "bias": np.ascontiguousarray(
                np.stack([bq[lo:hi], bk[lo:hi], bv[lo:hi]], axis=1)
            ).astype(np.float32),
            "cos": cosT if c == 0 else cos_id,
            "sin": sinT if c == 0 else sin_id,
            "rmat": rmat,
        })
    return in_maps


def run_device(inputs, trace=False, **kw):
    nc = _get_nc()
    in_maps = shard_inputs(**inputs)
    res = run_bass_kernel_spmd(nc, in_maps, core_ids=list(range(N_CORES)),
                               trace=trace, **kw)
    return res


def gather(res, bo):
    acc = res.results[0]["out"].astype(np.float32).copy()
    for c in range(1, N_CORES):
        acc += res.results[c]["out"]
    acc += bo[None, :].astype(np.float32)
    return acc.reshape(2, NB, HID)


def kernel(**inputs):
    res = run_device(inputs, trace=False)
    return gather(res, np.asarray(inputs["bo"], np.float32))


# revision 24
# speedup vs baseline: 1.1132x; 1.0145x over previous
"""Distributed Trainium2 kernel for nn_Attention_14697378086932.

Head-sharded (tensor-parallel) multi-head attention over 8 NeuronCores:
each core computes 2 of the 16 heads end-to-end.

Per core c:
  - QKV projections:  Q^T = Wq_c^T @ X^T  (f32r matmuls, contraction on
    hidden dim), giving Q^T/K^T/V^T in [128 local channels, 4096 tokens]
    layout (d-on-partitions), which is exactly the layout the scores
    matmul wants.
  - rotary: only global channels 0..63 are rotated (reference quirk), i.e.
    local channels 0..63 of core 0.  All cores run the same graph; cores
    1..7 receive cos=1/sin=0 so their "rotation" is the identity.
    rotate_half is a fixed 64x64 permutation matrix applied on the PE.
  - attention (per batch, per local head, flash-style over 128-token key
    chunks): S^T = K Q^T (f32r), P^T = exp(S^T) on the scalar engine
    (no max subtraction -- logits are bounded, f32/bf16 exp is safe),
    O^T = [V | 1]^T @ P^T (bf16) which yields the softmax denominator as
    a free 65th row.  Normalize with a reciprocal multiply.
  - output projection: partial = O_loc @ Wo_c (bf16), DMA'd out per
    128-token chunk.
Host sums the 8 partial outputs and adds bo.
"""
import sys
import types

sys.path.insert(0, "/opt/trn_rl_repo")

import numpy as np
import ml_dtypes

import concourse.bass as bass
import concourse.mybir as mybir
from concourse import bacc
from concourse.bass import ts, ds
from concourse.tile import TileContext
from concourse.masks import make_identity
from concourse.bass_utils import run_bass_kernel_spmd

F32 = mybir.dt.float32
F32R = mybir.dt.float32r
BF16 = mybir.dt.bfloat16

P = 128          # partitions / local channels per core
HID = 1024       # hidden
NT = 4096        # total tokens (batch 2 x 2048)
NB = 2048        # tokens per batch
HD = 64          # head dim
N_CORES = 8

_NC_CACHE = None


def build_nc():
    nc = bacc.Bacc("TRN2")

    xt = nc.declare_dram_parameter("xt", [HID, NT], F32R, isOutput=False)
    wq = nc.declare_dram_parameter("wq", [HID, P], F32R, isOutput=False)
    wk = nc.declare_dram_parameter("wk", [HID, P], F32R, isOutput=False)
    wv = nc.declare_dram_parameter("wv", [HID, P], F32R, isOutput=False)
    wo = nc.declare_dram_parameter("wo", [P, HID], BF16, isOutput=False)
    bia = nc.declare_dram_parameter("bias", [P, 3], F32, isOutput=False)
    cos = nc.declare_dram_parameter("cos", [HD, NT], F32, isOutput=False)
    sin = nc.declare_dram_parameter("sin", [HD, NT], F32, isOutput=False)
    rmat = nc.declare_dram_parameter("rmat", [HD, HD], F32R, isOutput=False)
    out = nc.declare_dram_parameter("out", [NT, HID], F32, isOutput=True)

    xt_r = xt[:].rearrange("(o p) n -> p o n", p=P)      # [128, 8, 4096]
    wq_r = wq[:].rearrange("(o p) m -> p o m", p=P)      # [128, 8, 128]
    wk_r = wk[:].rearrange("(o p) m -> p o m", p=P)
    wv_r = wv[:].rearrange("(o p) m -> p o m", p=P)

    with TileContext(nc) as tc:
        with tc.tile_pool(name="consts", bufs=1) as consts, \
             tc.tile_pool(name="big", bufs=1) as big:
            wqs = consts.tile([P, 8, P], F32R)
            wks = consts.tile([P, 8, P], F32R)
            wvs = consts.tile([P, 8, P], F32R)
            nc.sync.dma_start(wqs, wq_r)
            nc.sync.dma_start(wks, wk_r)
            nc.sync.dma_start(wvs, wv_r)
            wos = consts.tile([P, HID], BF16)
            nc.sync.dma_start(wos, wo[:])
            bias_t = consts.tile([P, 3], F32)
            nc.sync.dma_start(bias_t, bia[:])
            rmat_t = consts.tile([HD, HD], F32R)
            nc.sync.dma_start(rmat_t, rmat[:])
            ident = consts.tile([P, P], F32)
            make_identity(nc, ident)

            Qt = big.tile([P, NT], F32R)     # Q^T (local channels x tokens)
            Kt = big.tile([P, NT], F32R)
            Vt = big.tile([P, NT], F32)      # V^T, pre-transpose
            Ot = big.tile([P, NT], BF16)     # normalized attention out^T
            # V in natural [token, channel] layout + ones column, per head,
            # per 128-token key chunk: [128 tok, 32 chunks, 64 V | 1 | pad]
            VaugA = big.tile([P, 32, 66], BF16)
            VaugB = big.tile([P, 32, 66], BF16)
            nc.vector.memset(VaugA[:, :, 64:66], 1.0)
            nc.vector.memset(VaugB[:, :, 64:66], 1.0)

            # ---------------- Phase A: QKV projections + rope + V transpose
            # Token chunks are processed in pairs with the hidden-chunk (o)
            # loop outside the pair: consecutive matmuls then share their
            # stationary operand and the second skips its LDWEIGHTS.  Six
            # accumulators (3 projections x 2 token chunks) + rope + trans
            # use all 8 PSUM banks.
            with tc.tile_pool(name="xtp", bufs=4) as xtp, \
                 tc.tile_pool(name="ropet", bufs=2) as ropet, \
                 tc.tile_pool(name="trig", bufs=1) as trig, \
                 tc.tile_pool(name="psA", bufs=1, space="PSUM") as psA:
                cos_t = trig.tile([HD, NT], F32)
                sin_t = trig.tile([HD, NT], F32)
                nc.sync.dma_start(cos_t, cos[:])
                nc.sync.dma_start(sin_t, sin[:])
                for g in range(4):    # pairs of 512-token chunks
                    xtts = []
                    for u in range(2):
                        xtt = xtp.tile([P, 8, 512], F32R, tag="xt")
                        nc.sync.dma_start(
                            xtt, xt_r[:, :, ts(2 * g + u, 512)])
                        xtts.append(xtt)
                    for wt, bidx, dst in ((wqs, 0, Qt), (wks, 1, Kt),
                                          (wvs, 2, Vt)):
                        pss = [psA.tile([P, 512], F32, tag=f"ps{bidx}{u}",
                                        name=f"ps{bidx}{u}")
                               for u in range(2)]
                        for o in range(8):
                            for u in range(2):
                                nc.tensor.matmul(pss[u], wt[:, o],
                                                 xtts[u][:, o],
                                                 start=(o == 0), stop=(o == 7))
                        for u in range(2):
                            nc.scalar.activation(
                                dst[:, ts(2 * g + u, 512)], pss[u],
                                mybir.ActivationFunctionType.Identity,
                                bias=bias_t[:, bidx:bidx + 1])
                    # rope on first 64 local channels of Q and K
                    for u in range(2):
                        sl = ts(2 * g + u, 512)
                        for t in (Qt, Kt):
                            psr = psA.tile([P, 512], F32, tag="rope")
                            nc.tensor.matmul(psr[0:HD], rmat_t,
                                             t[0:HD, sl],
                                             start=True, stop=True)
                            tmp = ropet.tile([HD, 512], F32, tag="tmp")
                            nc.vector.tensor_tensor(
                                tmp, psr[0:HD], sin_t[:, sl],
                                mybir.AluOpType.mult)
                            nc.vector.tensor_tensor(
                                t[0:HD, sl], t[0:HD, sl],
                                cos_t[:, sl], mybir.AluOpType.mult)
                            nc.vector.tensor_tensor(
                                t[0:HD, sl], t[0:HD, sl], tmp,
                                mybir.AluOpType.add)
                        # V transpose into per-head layout (+ones col)
                        for s in range(4):
                            kc = (2 * g + u) * 4 + s
                            pst = psA.tile([P, 512], F32, tag="tr")
                            nc.tensor.transpose(pst[:, 0:P], Vt[:, ts(kc, P)],
                                                ident)
                            nc.vector.tensor_copy(VaugA[:, kc, 0:HD],
                                                  pst[:, 0:HD])
                            nc.vector.tensor_copy(VaugB[:, kc, 0:HD],
                                                  pst[:, HD:P])

            # ---------------- Phase B: attention + output projection
            # One local head per block, S double-buffered so the exp stream
            # on ScalarE (the bottleneck) never waits on S^T latency.  The
            # output projection of a finished (b, nqb) token range runs in a
            # dedicated psum pool and is INJECTED into the middle of the
            # following blocks, well after its normalize has completed, so
            # it never stalls the PE FIFO.
            with tc.tile_pool(name="ptp", bufs=4) as ptp, \
                 tc.tile_pool(name="osb", bufs=3) as osb, \
                 tc.tile_pool(name="nrm", bufs=2) as nrm, \
                 tc.tile_pool(name="spS", bufs=2, space="PSUM") as spS, \
                 tc.tile_pool(name="spO", bufs=1, space="PSUM") as spO, \
                 tc.tile_pool(name="spP", bufs=1, space="PSUM") as spP:

                def oproj_tile(q0, tch):
                    # output projection of one 128-token chunk (both heads)
                    t0 = q0 + tch * P
                    Pps = spP.tile([P, 1024], F32, tag="oproj")
                    for hf in range(2):
                        nc.tensor.matmul(
                            Pps[:, ts(hf, 512)],
                            Ot[:, t0:t0 + P],
                            wos[:, ts(hf, 512)],
                            start=True, stop=True)
                    ost = osb.tile([P, HID], F32, tag="ost")
                    nc.any.tensor_copy(ost, Pps)
                    nc.sync.dma_start(out[t0:t0 + P, :], ost)

                # (q0, tch) work items for output projection, produced as
                # blocks complete, consumed at injection points
                oproj_queue = []
                blocks = [(b, nqb, h)
                          for b in range(2) for nqb in range(2)
                          for h in range(2)]
                for bi, (b, nqb, h) in enumerate(blocks):
                    q0 = b * NB + nqb * 1024
                    hlo = h * HD
                    Vaug = VaugA if h == 0 else VaugB
                    Ops = spO.tile([HD + 1, 1024], F32, tag="O")

                    def s_exp(i):
                        k0 = b * NB + i * P
                        Sps = spS.tile([P, 1024], F32, tag="S")
                        for hf in range(2):
                            nc.tensor.matmul(
                                Sps[:, ts(hf, 512)],
                                Kt[hlo:hlo + HD, k0:k0 + P],
                                Qt[hlo:hlo + HD, ds(q0 + hf * 512, 512)],
                                start=True, stop=True)
                        Pt = ptp.tile([P, 1024], BF16, tag="P")
                        nc.scalar.activation(
                            Pt, Sps, mybir.ActivationFunctionType.Exp)
                        return Pt

                    def pv(i, Pt):
                        kc = b * 16 + i
                        for hf in range(2):
                            nc.tensor.matmul(
                                Ops[:, ts(hf, 512)],
                                Vaug[:, kc, 0:HD + 1],
                                Pt[:, ts(hf, 512)],
                                start=(i == 0), stop=(i == 15),
                                skip_group_check=True)

                    # depth-2 software pipeline; inject one oproj chunk
                    # every other chunk once the source tokens' normalize
                    # is safely in the past (>= 2 blocks = 32 chunks ago)
                    pend = []
                    for i in range(16):
                        if i >= 2:
                            pv(i - 2, pend.pop(0))
                        pend.append(s_exp(i))
                        if i % 2 == 0 and oproj_queue and bi >= 2:
                            src = oproj_queue[0]
                            if src[2] <= bi - 2:
                                oproj_queue.pop(0)
                                oproj_tile(src[0], src[1])
                    pv(14, pend.pop(0))
                    pv(15, pend.pop(0))

                    # normalize: copy out of PSUM right away so the O bank
                    # frees up, then rows 0..63 / row 64 from SBUF
                    osum = nrm.tile([HD + 1, 1024], F32, tag="osum")
                    nc.vector.tensor_copy(osum, Ops)
                    rc = nrm.tile([1, 1024], F32, tag="rc")
                    nc.vector.reciprocal(rc, osum[HD:HD + 1, :])
                    rcb = nrm.tile([HD, 1024], F32, tag="rcb")
                    nc.gpsimd.partition_broadcast(rcb, rc)
                    nc.vector.tensor_tensor(
                        Ot[hlo:hlo + HD, ds(q0, 1024)],
                        osum[0:HD, :],
                        rcb,
                        mybir.AluOpType.mult)
                    if h == 1:
                        for tch in range(8):
                            oproj_queue.append((q0, tch, bi))
                # drain remaining output-projection work
                for q0_, tch_, _ in oproj_queue:
                    oproj_tile(q0_, tch_)

    nc.compile()
    return nc


def _get_nc():
    global _NC_CACHE
    if _NC_CACHE is None:
        _NC_CACHE = build_nc()
    return _NC_CACHE


def shard_inputs(x, rope_cos, rope_sin, Wq, bq, Wk, bk, Wv, bv, Wo, bo):
    """Build per-core input maps."""
    xt = np.ascontiguousarray(x.reshape(NT, HID).T).astype(np.float32)
    cosT = np.ascontiguousarray(rope_cos.reshape(NT, HD).T).astype(np.float32)
    sinT = np.ascontiguousarray(rope_sin.reshape(NT, HD).T).astype(np.float32)
    cos_id = np.ones((HD, NT), np.float32)
    sin_id = np.zeros((HD, NT), np.float32)
    # rotate_half as matrix R: out = R @ t, R[2i,2i+1]=-1, R[2i+1,2i]=+1.
    # matmul computes lhsT.T @ rhs, so pass R.T.
    R = np.zeros((HD, HD), np.float32)
    idx = np.arange(0, HD, 2)
    R[idx, idx + 1] = -1.0
    R[idx + 1, idx] = 1.0
    rmat = np.ascontiguousarray(R.T)

    in_maps = []
    for c in range(N_CORES):
        lo, hi = c * P, (c + 1) * P
        in_maps.append({
            "xt": xt,
            "wq": np.ascontiguousarray(Wq[:, lo:hi]).astype(np.float32),
            "wk": np.ascontiguousarray(Wk[:, lo:hi]).astype(np.float32),
            "wv": np.ascontiguousarray(Wv[:, lo:hi]).astype(np.float32),
            "wo": np.ascontiguousarray(Wo[lo:hi, :]).astype(ml_dtypes.bfloat16),
            "bias": np.ascontiguousarray(
                np.stack([bq[lo:hi], bk[lo:hi], bv[lo:hi]], axis=1)
            ).astype(np.float32),
            "cos": cosT if c == 0 else cos_id,
            "sin": sinT if c == 0 else sin_id,
            "rmat": rmat,
        })
    return in_maps


def run_device(inputs, trace=False, **kw):
    nc = _get_nc()
    in_maps = shard_inputs(**inputs)
    res = run_bass_kernel_spmd(nc, in_maps, core_ids=list(range(N_CORES)),
                               trace=trace, **kw)
    return res


def gather(res, bo):
    acc = res.results[0]["out"].astype(np.float32).copy()
    for c in range(1, N_CORES):
        acc += res.results[c]["out"]
    acc += bo[None, :].astype(np.float32)
    return acc.reshape(2, NB, HID)


def kernel(**inputs):
    res = run_device(inputs, trace=False)
    return gather(res, np.asarray(inputs["bo"], np.float32))


# revision 26
# speedup vs baseline: 1.1931x; 1.0718x over previous
"""Distributed Trainium2 kernel for nn_Attention_14697378086932.

Head-sharded (tensor-parallel) multi-head attention over 8 NeuronCores:
each core computes 2 of the 16 heads end-to-end.

Per core c:
  - QKV projections:  Q^T = Wq_c^T @ X^T  (f32r matmuls, contraction on
    hidden dim), giving Q^T/K^T/V^T in [128 local channels, 4096 tokens]
    layout (d-on-partitions), which is exactly the layout the scores
    matmul wants.
  - rotary: only global channels 0..63 are rotated (reference quirk), i.e.
    local channels 0..63 of core 0.  All cores run the same graph; cores
    1..7 receive cos=1/sin=0 so their "rotation" is the identity.
    rotate_half is a fixed 64x64 permutation matrix applied on the PE.
  - attention (per batch, per local head, flash-style over 128-token key
    chunks): S^T = K Q^T (f32r), P^T = exp(S^T) on the scalar engine
    (no max subtraction -- logits are bounded, f32/bf16 exp is safe),
    O^T = [V | 1]^T @ P^T (bf16) which yields the softmax denominator as
    a free 65th row.  Normalize with a reciprocal multiply.
  - output projection: partial = O_loc @ Wo_c (bf16), DMA'd out per
    128-token chunk.
Host sums the 8 partial outputs and adds bo.
"""
import sys
import types

sys.path.insert(0, "/opt/trn_rl_repo")

import numpy as np
import ml_dtypes

import concourse.bass as bass
import concourse.mybir as mybir
from concourse import bacc
from concourse.bass import ts, ds
from concourse.tile import TileContext
from concourse.masks import make_identity
from concourse.bass_utils import run_bass_kernel_spmd

F32 = mybir.dt.float32
F32R = mybir.dt.float32r
BF16 = mybir.dt.bfloat16

P = 128          # partitions / local channels per core
HID = 1024       # hidden
NT = 4096        # total tokens (batch 2 x 2048)
NB = 2048        # tokens per batch
HD = 64          # head dim
N_CORES = 8

_NC_CACHE = None


def build_nc():
    nc = bacc.Bacc("TRN2")

    xt = nc.declare_dram_parameter("xt", [HID, NT], F32R, isOutput=False)
    wq = nc.declare_dram_parameter("wq", [HID, P], F32R, isOutput=False)
    wk = nc.declare_dram_parameter("wk", [HID, P], F32R, isOutput=False)
    wv = nc.declare_dram_parameter("wv", [HID, P], F32R, isOutput=False)
    wo = nc.declare_dram_parameter("wo", [P, HID], BF16, isOutput=False)
    bia = nc.declare_dram_parameter("bias", [P, 3], F32, isOutput=False)
    cos = nc.declare_dram_parameter("cos", [HD, NT], F32, isOutput=False)
    sin = nc.declare_dram_parameter("sin", [HD, NT], F32, isOutput=False)
    rmat = nc.declare_dram_parameter("rmat", [HD, HD], F32R, isOutput=False)
    out = nc.declare_dram_parameter("out", [NT, HID], F32, isOutput=True)

    xt_r = xt[:].rearrange("(o p) n -> p o n", p=P)      # [128, 8, 4096]
    wq_r = wq[:].rearrange("(o p) m -> p o m", p=P)      # [128, 8, 128]
    wk_r = wk[:].rearrange("(o p) m -> p o m", p=P)
    wv_r = wv[:].rearrange("(o p) m -> p o m", p=P)

    with TileContext(nc) as tc:
        with tc.tile_pool(name="consts", bufs=1) as consts, \
             tc.tile_pool(name="big", bufs=1) as big:
            wqs = consts.tile([P, 8, P], F32R)
            wks = consts.tile([P, 8, P], F32R)
            wvs = consts.tile([P, 8, P], F32R)
            nc.sync.dma_start(wqs, wq_r)
            nc.sync.dma_start(wks, wk_r)
            nc.sync.dma_start(wvs, wv_r)
            wos = consts.tile([P, HID], BF16)
            nc.sync.dma_start(wos, wo[:])
            bias_t = consts.tile([P, 3], F32)
            nc.sync.dma_start(bias_t, bia[:])
            rmat_t = consts.tile([HD, HD], F32R)
            nc.sync.dma_start(rmat_t, rmat[:])
            ident = consts.tile([P, P], F32)
            make_identity(nc, ident)

            Qt = big.tile([P, NT], F32R)     # Q^T (local channels x tokens)
            Kt = big.tile([P, NT], F32R)
            Vt = big.tile([P, NT], F32)      # V^T, pre-transpose
            Ot = big.tile([P, NT], BF16)     # normalized attention out^T
            # V in natural [token, channel] layout + ones column, per head,
            # per 128-token key chunk: [128 tok, 32 chunks, 64 V | 1 | pad]
            VaugA = big.tile([P, 32, 66], BF16)
            VaugB = big.tile([P, 32, 66], BF16)
            nc.vector.memset(VaugA[:, :, 64:66], 1.0)
            nc.vector.memset(VaugB[:, :, 64:66], 1.0)

            # ---------------- Phase A: QKV projections + rope + V transpose
            # Token chunks are processed in pairs with the hidden-chunk (o)
            # loop outside the pair: consecutive matmuls then share their
            # stationary operand and the second skips its LDWEIGHTS.  Six
            # accumulators (3 projections x 2 token chunks) + rope + trans
            # use all 8 PSUM banks.
            with tc.tile_pool(name="xtp", bufs=4) as xtp, \
                 tc.tile_pool(name="ropet", bufs=2) as ropet, \
                 tc.tile_pool(name="trig", bufs=1) as trig, \
                 tc.tile_pool(name="psA", bufs=1, space="PSUM") as psA:
                cos_t = trig.tile([HD, NT], F32)
                sin_t = trig.tile([HD, NT], F32)
                nc.sync.dma_start(cos_t, cos[:])
                nc.sync.dma_start(sin_t, sin[:])
                for g in range(4):    # pairs of 512-token chunks
                    xtts = []
                    for u in range(2):
                        xtt = xtp.tile([P, 8, 512], F32R, tag="xt")
                        nc.sync.dma_start(
                            xtt, xt_r[:, :, ts(2 * g + u, 512)])
                        xtts.append(xtt)
                    for wt, bidx, dst in ((wqs, 0, Qt), (wks, 1, Kt),
                                          (wvs, 2, Vt)):
                        pss = [psA.tile([P, 512], F32, tag=f"ps{bidx}{u}",
                                        name=f"ps{bidx}{u}")
                               for u in range(2)]
                        for o in range(8):
                            for u in range(2):
                                nc.tensor.matmul(pss[u], wt[:, o],
                                                 xtts[u][:, o],
                                                 start=(o == 0), stop=(o == 7))
                        for u in range(2):
                            nc.scalar.activation(
                                dst[:, ts(2 * g + u, 512)], pss[u],
                                mybir.ActivationFunctionType.Identity,
                                bias=bias_t[:, bidx:bidx + 1])
                    # rope on first 64 local channels of Q and K
                    for u in range(2):
                        sl = ts(2 * g + u, 512)
                        for t in (Qt, Kt):
                            psr = psA.tile([P, 512], F32, tag="rope")
                            nc.tensor.matmul(psr[0:HD], rmat_t,
                                             t[0:HD, sl],
                                             start=True, stop=True)
                            tmp = ropet.tile([HD, 512], F32, tag="tmp")
                            nc.vector.tensor_tensor(
                                tmp, psr[0:HD], sin_t[:, sl],
                                mybir.AluOpType.mult)
                            nc.vector.tensor_tensor(
                                t[0:HD, sl], t[0:HD, sl],
                                cos_t[:, sl], mybir.AluOpType.mult)
                            nc.vector.tensor_tensor(
                                t[0:HD, sl], t[0:HD, sl], tmp,
                                mybir.AluOpType.add)
                        # V transpose into per-head layout (+ones col)
                        for s in range(4):
                            kc = (2 * g + u) * 4 + s
                            pst = psA.tile([P, 512], F32, tag="tr")
                            nc.tensor.transpose(pst[:, 0:P], Vt[:, ts(kc, P)],
                                                ident)
                            nc.vector.tensor_copy(VaugA[:, kc, 0:HD],
                                                  pst[:, 0:HD])
                            nc.vector.tensor_copy(VaugB[:, kc, 0:HD],
                                                  pst[:, HD:P])

            # ---------------- Phase B: attention + output projection
            # One local head per block, S double-buffered so the exp stream
            # on ScalarE (the bottleneck) never waits on S^T latency.  The
            # output projection of a finished (b, nqb) token range runs in a
            # dedicated psum pool and is INJECTED into the middle of the
            # following blocks, well after its normalize has completed, so
            # it never stalls the PE FIFO.
            with tc.tile_pool(name="ptp", bufs=4) as ptp, \
                 tc.tile_pool(name="osb", bufs=3) as osb, \
                 tc.tile_pool(name="nrm", bufs=2) as nrm, \
                 tc.tile_pool(name="spS", bufs=2, space="PSUM") as spS, \
                 tc.tile_pool(name="spO", bufs=1, space="PSUM") as spO, \
                 tc.tile_pool(name="spP", bufs=1, space="PSUM") as spP, \
                 tc.tile_pool(name="spD", bufs=1, space="PSUM") as spD:

                # Keep-warm scratch: the PE clock-gate (HAM) re-throttles to
                # 1.2 GHz after any idle window, and block boundaries leave
                # short PE gaps that put the whole attention phase in a
                # cold-clock equilibrium.  Filler matmuls into this scratch
                # bank bridge those gaps so the PE stays at 2.4 GHz.
                dmy = spD.tile([P, 512], F32, tag="dummy")

                def keep_warm(n=1):
                    for _ in range(n):
                        nc.tensor.matmul(dmy, wos[:, 0:P], wos[:, 0:512],
                                         start=True, stop=True,
                                         skip_group_check=True)

                def oproj_tile(q0, tch):
                    # output projection of one 128-token chunk (both heads)
                    t0 = q0 + tch * P
                    ost = osb.tile([P, HID], F32, tag="ost")
                    for hf in range(2):
                        Pps = spP.tile([P, 512], F32, tag="oproj")
                        nc.tensor.matmul(
                            Pps,
                            Ot[:, t0:t0 + P],
                            wos[:, ts(hf, 512)],
                            start=True, stop=True)
                        nc.any.tensor_copy(ost[:, ts(hf, 512)], Pps)
                    nc.sync.dma_start(out[t0:t0 + P, :], ost)

                # (q0, tch) work items for output projection, produced as
                # blocks complete, consumed at injection points
                oproj_queue = []
                blocks = [(b, nqb, h)
                          for b in range(2) for nqb in range(2)
                          for h in range(2)]
                for bi, (b, nqb, h) in enumerate(blocks):
                    q0 = b * NB + nqb * 1024
                    hlo = h * HD
                    Vaug = VaugA if h == 0 else VaugB
                    Ops = spO.tile([HD + 1, 1024], F32, tag="O")

                    def s_exp(i):
                        k0 = b * NB + i * P
                        Sps = spS.tile([P, 1024], F32, tag="S")
                        for hf in range(2):
                            nc.tensor.matmul(
                                Sps[:, ts(hf, 512)],
                                Kt[hlo:hlo + HD, k0:k0 + P],
                                Qt[hlo:hlo + HD, ds(q0 + hf * 512, 512)],
                                start=True, stop=True)
                        Pt = ptp.tile([P, 1024], BF16, tag="P")
                        nc.scalar.activation(
                            Pt, Sps, mybir.ActivationFunctionType.Exp)
                        return Pt

                    def pv(i, Pt):
                        kc = b * 16 + i
                        for hf in range(2):
                            nc.tensor.matmul(
                                Ops[:, ts(hf, 512)],
                                Vaug[:, kc, 0:HD + 1],
                                Pt[:, ts(hf, 512)],
                                start=(i == 0), stop=(i == 15),
                                skip_group_check=True)

                    # depth-2 software pipeline; inject one oproj chunk
                    # every other chunk once the source tokens' normalize
                    # is safely in the past (>= 2 blocks = 32 chunks ago)
                    pend = []
                    for i in range(16):
                        if i >= 2:
                            pv(i - 2, pend.pop(0))
                        pend.append(s_exp(i))
                        if i % 2 == 0 and oproj_queue and bi >= 2:
                            src = oproj_queue[0]
                            if src[2] <= bi - 2:
                                oproj_queue.pop(0)
                                oproj_tile(src[0], src[1])
                    pv(14, pend.pop(0))
                    pv(15, pend.pop(0))
                    keep_warm(2)

                    # normalize: copy out of PSUM right away so the O bank
                    # frees up, then rows 0..63 / row 64 from SBUF
                    osum = nrm.tile([HD + 1, 1024], F32, tag="osum")
                    nc.vector.tensor_copy(osum, Ops)
                    rc = nrm.tile([1, 1024], F32, tag="rc")
                    nc.vector.reciprocal(rc, osum[HD:HD + 1, :])
                    rcb = nrm.tile([HD, 1024], F32, tag="rcb")
                    nc.gpsimd.partition_broadcast(rcb, rc)
                    nc.vector.tensor_tensor(
                        Ot[hlo:hlo + HD, ds(q0, 1024)],
                        osum[0:HD, :],
                        rcb,
                        mybir.AluOpType.mult)
                    keep_warm(2)
                    if h == 1:
                        for tch in range(8):
                            oproj_queue.append((q0, tch, bi))
                # drain remaining output-projection work
                for q0_, tch_, _ in oproj_queue:
                    oproj_tile(q0_, tch_)
                # keep the scratch tile alive past DCE
                sink = nrm.tile([P, 1], F32, tag="sink")
                nc.vector.tensor_copy(sink, dmy[:, 0:1])

    nc.compile()
    return nc


def _get_nc():
    global _NC_CACHE
    if _NC_CACHE is None:
        _NC_CACHE = build_nc()
    return _NC_CACHE


def shard_inputs(x, rope_cos, rope_sin, Wq, bq, Wk, bk, Wv, bv, Wo, bo):
    """Build per-core input maps."""
    xt = np.ascontiguousarray(x.reshape(NT, HID).T).astype(np.float32)
    cosT = np.ascontiguousarray(rope_cos.reshape(NT, HD).T).astype(np.float32)
    sinT = np.ascontiguousarray(rope_sin.reshape(NT, HD).T).astype(np.float32)
    cos_id = np.ones((HD, NT), np.float32)
    sin_id = np.zeros((HD, NT), np.float32)
    # rotate_half as matrix R: out = R @ t, R[2i,2i+1]=-1, R[2i+1,2i]=+1.
    # matmul computes lhsT.T @ rhs, so pass R.T.
    R = np.zeros((HD, HD), np.float32)
    idx = np.arange(0, HD, 2)
    R[idx, idx + 1] = -1.0
    R[idx + 1, idx] = 1.0
    rmat = np.ascontiguousarray(R.T)

    in_maps = []
    for c in range(N_CORES):
        lo, hi = c * P, (c + 1) * P
        in_maps.append({
            "xt": xt,
            "wq": np.ascontiguousarray(Wq[:, lo:hi]).astype(np.float32),
            "wk": np.ascontiguousarray(Wk[:, lo:hi]).astype(np.float32),
            "wv": np.ascontiguousarray(Wv[:, lo:hi]).astype(np.float32),
            "wo": np.ascontiguousarray(Wo[lo:hi, :]).astype(ml_dtypes.bfloat16),
            "bias": np.ascontiguousarray(
                np.stack([bq[lo:hi], bk[lo:hi], bv[lo:hi]], axis=1)
            ).astype(np.float32),
            "cos": cosT if c == 0 else cos_id,
            "sin": sinT if c == 0 else sin_id,
            "rmat": rmat,
        })
    return in_maps


def run_device(inputs, trace=False, **kw):
    nc = _get_nc()
    in_maps = shard_inputs(**inputs)
    res = run_bass_kernel_spmd(nc, in_maps, core_ids=list(range(N_CORES)),
                               trace=trace, **kw)
    return res


def gather(res, bo):
    acc = res.results[0]["out"].astype(np.float32).copy()
    for c in range(1, N_CORES):
        acc += res.results[c]["out"]
    acc += bo[None, :].astype(np.float32)
    return acc.reshape(2, NB, HID)


def kernel(**inputs):
    res = run_device(inputs, trace=False)
    return gather(res, np.asarray(inputs["bo"], np.float32))
